# revision 1
# baseline (speedup 1.0000x reference)
"""Trainium2 Bass kernel for the KalmanFilter linear recurrence.

  x = data - mean;  z0 = R @ x[0];  drive = inputs @ C.T
  z_{t+1} = A z_t + drive[t]   (T = 32768 steps, dim 512)
  result  = Z[1:] @ B.T + mean

Strategy (8 NeuronCores, sequence-parallel, no collectives):
  - ||A^k|| decays like 0.9^k (spectral radius 0.9), so the recurrence
    forgets its state after H=128 steps to ~1e-5 relative (far
    below the TF32 matmul noise this kernel runs at).
  - Each core owns 4096 contiguous steps, split into 256 chunks of S=16
    steps + K=8 extra "halo" chunks covering the preceding H=128 steps.
  - Phase A: batched zero-init scan over all 268 chunks (state tiles
    [512, 264], 15 matmul steps) -> per-chunk accumulated drives b_c.
  - Phase B: chunk-start states w_c = sum_{p=0}^{K-1} (A^16)^p b_{c-1-p}
    (banded combine; truncated at ||A^128|| ~ 4e-4 of a unit).
    Taps p>=1 use host-precomputed (A^16)^p in bf16 (their contribution
    is scaled by ||A^{16p}|| <= 0.8, so bf16 error is ~1e-4 relative).
  - Phase C: re-scan the 256 real chunks from inits w_c; each step also
    applies the output projection B.T and streams rows to DRAM.
  - z0 only affects output rows 0..H-1 (through A^n z0); that correction
    is added on the host, so the device never sees `data`/`R`.
  All matmuls run as float32r (TF32: ~1e-4 relative, fp32 accumulate).
"""
import numpy as np
import concourse.bacc as bacc
import concourse.mybir as mybir
from concourse import tile
from concourse.bass_utils import run_bass_kernel_spmd

T = 32768
DZ = 512
DU = 256
NCORE = 8
TLOC = T // NCORE          # 4096
S = 16                     # steps per chunk
BCH = TLOC // S            # 256 chunks per core
H = 128                    # halo steps (forgetting horizon)
K = H // S                 # 8 banded taps (incl. identity)
NCH = BCH + K              # 268 chunks in phase A
ULEN = TLOC + H            # 4288 drive rows per core
UPAD = ((ULEN + 127) // 128) * 128   # padded to a multiple of 128
NTB = UPAD // 128          # row-tiles of u

f32 = mybir.dt.float32
f32r = mybir.dt.float32r
bf16 = mybir.dt.bfloat16

_CACHE = {}


def _emit(nc):
    u_d = nc.dram_tensor("u", (UPAD, DU), f32, kind="ExternalInput")
    at_d = nc.dram_tensor("at", (DZ, DZ), f32r, kind="ExternalInput")
    ct_d = nc.dram_tensor("ct", (DU, DZ), f32r, kind="ExternalInput")
    bt_d = nc.dram_tensor("bt", (DZ, DZ), f32r, kind="ExternalInput")
    mb_d = nc.dram_tensor("mb", (K - 1, 128, 4, DZ), bf16, kind="ExternalInput")
    mn_d = nc.dram_tensor("mn", (128, DZ), f32, kind="ExternalInput")
    id_d = nc.dram_tensor("id", (128, 128), f32, kind="ExternalInput")
    out_d = nc.dram_tensor("out", (TLOC, DZ), f32, kind="ExternalOutput")

    with tile.TileContext(nc) as tc:
        with tc.tile_pool(name="const", bufs=1) as cpool, \
             tc.tile_pool(name="dt", bufs=1) as dpool, \
             tc.tile_pool(name="ustg", bufs=4) as upool, \
             tc.tile_pool(name="utb", bufs=3) as utpool, \
             tc.tile_pool(name="mb", bufs=7) as mbpool, \
             tc.tile_pool(name="st", bufs=2) as stpool, \
             tc.tile_pool(name="ob", bufs=4) as opool, \
             tc.tile_pool(name="ps", bufs=8, space="PSUM") as pp:

            # ---- constant loads ----
            at_sb = [cpool.tile([128, DZ], f32r, tag=f"at{k}", name=f"at{k}") for k in range(4)]
            ct_sb = [cpool.tile([128, DZ], f32r, tag=f"ct{k}", name=f"ct{k}") for k in range(2)]
            bt_sb = [cpool.tile([128, DZ], f32r, tag=f"bt{k}", name=f"bt{k}") for k in range(4)]
            mn_sb = cpool.tile([128, DZ], f32, tag="mn")
            id_sb = cpool.tile([128, 128], f32, tag="id")
            for k in range(4):
                nc.sync.dma_start(at_sb[k][:], at_d[128 * k:128 * (k + 1), :])
                nc.sync.dma_start(bt_sb[k][:], bt_d[128 * k:128 * (k + 1), :])
            for k in range(2):
                nc.sync.dma_start(ct_sb[k][:], ct_d[128 * k:128 * (k + 1), :])
            nc.sync.dma_start(mn_sb[:], mn_d[:])
            nc.sync.dma_start(id_sb[:], id_d[:])

            # drive rows (transposed): dT[m] holds drive.T[128m:128(m+1), :]
            dt_sb = [dpool.tile([128, UPAD], f32r, tag=f"dt{m}", name=f"dt{m}") for m in range(4)]

            # ---- transpose u + drive matmul, streamed over n-blocks ----
            for nb in range((UPAD + 511) // 512):   # blocks of <=512 drive cols
                nb0 = nb * 512
                w = min(512, UPAD - nb0)
                utb = utpool.tile([128, 1024], f32r, tag="utb")
                for sub in range(w // 128):         # row-tiles of u in this block
                    tb = nb * 4 + sub
                    stg = upool.tile([128, DU], f32, tag="ustg")
                    nc.sync.dma_start(stg[:], u_d[128 * tb:128 * (tb + 1), :])
                    for kk in range(2):
                        pst = pp.tile([128, 128], f32, tag="ps")
                        nc.tensor.transpose(
                            pst[:], stg[:, 128 * kk:128 * (kk + 1)], id_sb[:])
                        nc.any.tensor_copy(
                            utb[:, 512 * kk + 128 * sub:512 * kk + 128 * sub + 128],
                            pst[:])
                for m in range(4):
                    psd = pp.tile([128, 512], f32, tag="ps")
                    for kk in range(2):
                        nc.tensor.matmul(
                            psd[:, :w],
                            ct_sb[kk][:, 128 * m:128 * (m + 1)],
                            utb[:, 512 * kk:512 * kk + w],
                            start=(kk == 0), stop=(kk == 1))
                    nc.any.tensor_copy(dt_sb[m][:, nb0:nb0 + w], psd[:, :w])

            # ---- phase A: zero-init scan over NCH chunks ----
            bmat = [cpool.tile([128, NCH], f32r, tag=f"bm{m}", name=f"bm{m}") for m in range(4)]
            st_prev = []
            for m in range(4):
                t0 = stpool.tile([128, NCH], f32r, tag=f"st{m}", name=f"st0_{m}")
                nc.vector.tensor_copy(
                    t0[:], dt_sb[m][:, 0:16 * NCH:16].bitcast(f32))
                st_prev.append(t0)
            for k in range(1, S):
                psl = [pp.tile([128, NCH], f32, tag="ps", name=f"psA{k}_{_m}") for _m in range(4)]
                for m in range(4):
                    for kk in range(4):
                        nc.tensor.matmul(
                            psl[m][:],
                            at_sb[kk][:, 128 * m:128 * (m + 1)],
                            st_prev[kk][:],
                            start=(kk == 0), stop=(kk == 3))
                st_new = []
                for m in range(4):
                    dst = (bmat[m] if k == S - 1 else
                           stpool.tile([128, NCH], f32r, tag=f"st{m}", name=f"stA{k}_{m}"))
                    nc.vector.tensor_tensor(
                        dst[:], psl[m][:],
                        dt_sb[m][:, k:k + 16 * (NCH - 1) + 1:16].bitcast(f32),
                        op=mybir.AluOpType.add)
                    st_new.append(dst)
                st_prev = st_new

            # bf16 copy of b for the banded taps
            bm16 = [cpool.tile([128, NCH], bf16, tag=f"bh{m}", name=f"bh{m}") for m in range(4)]
            for m in range(4):
                nc.vector.tensor_copy(bm16[m][:], bmat[m][:].bitcast(f32))

            # ---- phase B: banded combine  w_c = sum_p M_p b_{c-1-p} ----
            psw = [pp.tile([128, BCH], f32, tag="ps", name=f"psW{_m}") for _m in range(4)]
            for p in range(1, K):
                mbt = mbpool.tile([128, 4 * DZ], bf16, tag="mbt")
                nc.sync.dma_start(
                    mbt[:].rearrange("p (k n) -> p k n", k=4), mb_d[p - 1])
                lo = K - 1 - p
                for m in range(4):
                    for kk in range(4):
                        nc.tensor.matmul(
                            psw[m][:],
                            mbt[:, 512 * kk + 128 * m:512 * kk + 128 * m + 128],
                            bm16[kk][:, lo:lo + BCH],
                            start=(p == 1 and kk == 0),
                            stop=(p == K - 1 and kk == 3))
            w_sb = []
            for m in range(4):
                wt = cpool.tile([128, BCH], f32r, tag=f"w{m}", name=f"w{m}")
                nc.vector.tensor_tensor(
                    wt[:], psw[m][:], bmat[m][:, K - 1:K - 1 + BCH].bitcast(f32),
                    op=mybir.AluOpType.add)
                w_sb.append(wt)

            # ---- phase C: scan 256 chunks from w_c, fused output proj ----
            st_prev = w_sb
            for k in range(S):
                psl = [pp.tile([128, BCH], f32, tag="ps", name=f"psC{k}_{_m}") for _m in range(4)]
                for m in range(4):
                    for kk in range(4):
                        nc.tensor.matmul(
                            psl[m][:],
                            at_sb[kk][:, 128 * m:128 * (m + 1)],
                            st_prev[kk][:],
                            start=(kk == 0), stop=(kk == 3))
                st_new = []
                for m in range(4):
                    dst = stpool.tile([128, BCH], f32r, tag=f"sc{m}", name=f"stC{k}_{m}")
                    nc.vector.tensor_tensor(
                        dst[:], psl[m][:],
                        dt_sb[m][:, H + k:H + k + 16 * (BCH - 1) + 1:16].bitcast(f32),
                        op=mybir.AluOpType.add)
                    st_new.append(dst)
                st_prev = st_new
                # output rows t = 16*c + k for all 256 chunks c
                for h in range(2):
                    pso = pp.tile([128, DZ], f32, tag="ps")
                    for kk in range(4):
                        nc.tensor.matmul(
                            pso[:],
                            st_new[kk][:, 128 * h:128 * (h + 1)],
                            bt_sb[kk][:],
                            start=(kk == 0), stop=(kk == 3))
                    ob = opool.tile([128, DZ], f32, tag="ob")
                    nc.vector.tensor_tensor(
                        ob[:], pso[:], mn_sb[:], op=mybir.AluOpType.add)
                    r0 = 2048 * h + k
                    nc.sync.dma_start(out_d[r0:r0 + 2033:16, :], ob[:])
    nc.compile()
    return nc


def _build():
    if "nc" not in _CACHE:
        nc = bacc.Bacc("TRN2", target_bir_lowering=False, debug=False)
        _CACHE["nc"] = _emit(nc)
    return _CACHE["nc"]


def _host_prep(inputs_np, mean, A, B, C):
    A64 = A.astype(np.float64)
    AS = np.linalg.matrix_power(A64, S)
    mb = np.empty((K - 1, 128, 4, DZ), np.float32)
    Mp = AS.copy()
    for p in range(1, K):
        mt = Mp.T.astype(np.float32)        # lhsT layout: [z_in, z_out]
        mb[p - 1] = mt.reshape(4, 128, DZ).transpose(1, 0, 2)
        Mp = Mp @ AS
    import ml_dtypes
    mb = mb.astype(ml_dtypes.bfloat16)

    pad = np.zeros((H, DU), np.float32)
    up = np.concatenate([pad, inputs_np], axis=0)       # (T + H, DU)
    u_list = []
    for i in range(NCORE):
        ui = np.zeros((UPAD, DU), np.float32)
        ui[:ULEN] = up[i * TLOC:i * TLOC + ULEN]
        u_list.append(ui)

    shared = {
        "at": np.ascontiguousarray(A.T),
        "ct": np.ascontiguousarray(C.T),
        "bt": np.ascontiguousarray(B.T),
        "mb": mb,
        "mn": np.ascontiguousarray(np.broadcast_to(mean, (128, DZ))),
        "id": np.eye(128, dtype=np.float32),
    }
    return [{**shared, "u": u_list[i]} for i in range(NCORE)]


def kernel(data, inputs, mean, A, B, C, recognition_matrix, steps=None, **kw):
    data = np.asarray(data, np.float32)
    inputs_np = np.asarray(inputs, np.float32)
    mean = np.asarray(mean, np.float32)
    A = np.asarray(A, np.float32)
    B = np.asarray(B, np.float32)
    C = np.asarray(C, np.float32)
    R = np.asarray(recognition_matrix, np.float32)

    nc = _build()
    in_maps = _host_prep(inputs_np, mean, A, B, C)
    res = run_bass_kernel_spmd(nc, in_maps, list(range(NCORE)))
    out = np.concatenate([res.results[i]["out"] for i in range(NCORE)], axis=0)

    # host correction: output row n-1 += (A^n z0) @ B.T for n = 1..H
    z0 = (R.astype(np.float64) @ (data[0] - mean[0]).astype(np.float64))
    zc = z0
    A64, B64 = A.astype(np.float64), B.astype(np.float64)
    corr = np.empty((H, DZ), np.float64)
    for n in range(1, H + 1):
        zc = A64 @ zc
        corr[n - 1] = B64 @ zc
    out[:H] += corr.astype(np.float32)
    return out



# revision 4
# speedup vs baseline: 3.3124x; 3.3124x over previous
"""Trainium2 Bass kernel for the KalmanFilter linear recurrence.

  x = data - mean;  z0 = R @ x[0];  drive = inputs @ C.T
  z_{t+1} = A z_t + drive[t]   (T = 32768 steps, dim 512)
  result  = Z[1:] @ B.T + mean

Strategy (8 NeuronCores, sequence-parallel):
  - ||A^k|| decays like 0.9^k (spectral radius 0.9), so the recurrence
    forgets its state after H=128 steps to ~1e-5 relative.
  - Each core owns 4096 contiguous steps, split into 256 chunks of S=16
    steps + K=8 extra "halo" chunks covering the preceding H=128 steps.
  - Phase A: batched zero-init scan over all 268 chunks (state tiles
    [512, 268], 15 matmul steps) -> per-chunk accumulated drives b_c.
  - Phase B: chunk-start states w_c = sum_{p=0}^{K-1} (A^16)^p b_{c-1-p}
    (banded combine truncated at ||A^128|| ~ 4e-4 of a unit). The tap
    matrices (A^16)^p are computed ON DEVICE by repeated squaring in TF32.
  - Phase C: re-scan the 256 real chunks from inits w_c; each step also
    applies the output projection B.T and streams fp16 rows to DRAM.
  - z0 only affects output rows 0..H-1 (through A^n z0); that correction
    and the +mean are added on the host.

The end-to-end time is dominated by the ~45 MB/s axon host<->device pipe,
so the wire format is minimal:
  - u is shipped pre-transposed as int8 (absmax-scaled; the scale is
    folded into C.T on the host).
  - A.T / C.T / B.T / identity are shared by all cores: each core uploads
    only a 1/8 slice and an on-device AllGather collective rebuilds them.
  - the output returns as fp16; +mean happens on the host.
  - the PJRT output buffers live on device and are reused without
    donation (the kernel writes every output element, so stale contents
    are harmless and no host zeros ever cross the link).
Total wire traffic ~45 MB vs ~220 MB for the naive path.
"""
import numpy as np
import concourse.bacc as bacc
import concourse.mybir as mybir
from concourse import tile

T = 32768
DZ = 512
DU = 256
NCORE = 8
TLOC = T // NCORE          # 4096
S = 16                     # steps per chunk
BCH = TLOC // S            # 256 chunks per core
H = 128                    # halo steps (forgetting horizon)
K = H // S                 # 8 banded taps (incl. identity)
NCH = BCH + K              # 268 chunks in phase A
ULEN = TLOC + H            # 4288 drive rows per core
UPAD = ((ULEN + 127) // 128) * 128   # 4352, padded to a multiple of 128

f32 = mybir.dt.float32
f32r = mybir.dt.float32r
fp16 = mybir.dt.float16
i8 = mybir.dt.int8

_CACHE = {}


def _emit(nc):
    ut_d = nc.dram_tensor("ut", (DU, UPAD), i8, kind="ExternalInput")
    at_d = nc.dram_tensor("at", (DZ // NCORE, DZ), f32r, kind="ExternalInput")
    ct_d = nc.dram_tensor("ct", (DU // NCORE, DZ), fp16, kind="ExternalInput")
    bt_d = nc.dram_tensor("bt", (DZ // NCORE, DZ), fp16, kind="ExternalInput")
    id_d = nc.dram_tensor("id", (128 // NCORE, 128), f32, kind="ExternalInput")
    out_d = nc.dram_tensor("out", (TLOC, DZ), fp16, kind="ExternalOutput")

    with tile.TileContext(nc) as tc:
        with tc.tile_pool(name="const", bufs=1) as cpool, \
             tc.tile_pool(name="dt", bufs=1) as dpool, \
             tc.tile_pool(name="st", bufs=2) as stpool, \
             tc.tile_pool(name="ob", bufs=4) as opool, \
             tc.tile_pool(name="dram", bufs=1, space="DRAM") as drampool, \
             tc.tile_pool(name="ps", bufs=8, space="PSUM") as pp:

            # ---- AllGather shared constants (each core ships 1/8) ----
            rg = [list(range(NCORE))]
            byp = mybir.AluOpType.bypass
            ag = {}
            for nm, dram_in, shape, dty in (
                    ("at", at_d, (DZ, DZ), f32r),
                    ("ct", ct_d, (DU, DZ), fp16),
                    ("bt", bt_d, (DZ, DZ), fp16),
                    ("id", id_d, (128, 128), f32)):
                bi = drampool.tile([shape[0] // NCORE, shape[1]], dty,
                                   tag=f"agi_{nm}", name=f"agi_{nm}")
                bo = drampool.tile(list(shape), dty, tag=f"ago_{nm}",
                                   name=f"ago_{nm}")
                nc.gpsimd.dma_start(bi[:], dram_in[:])
                nc.gpsimd.collective_compute(
                    "AllGather", byp, replica_groups=rg,
                    ins=[bi.opt()], outs=[bo.opt()])
                ag[nm] = bo

            # ---- constant loads ----
            at_sb = [cpool.tile([128, DZ], f32r, tag=f"at{k}", name=f"at{k}") for k in range(4)]
            ct_sb = [cpool.tile([128, DZ], fp16, tag=f"ct{k}", name=f"ct{k}") for k in range(2)]
            bth = [cpool.tile([128, DZ], fp16, tag=f"bth{k}", name=f"bth{k}") for k in range(4)]
            bt_sb = [cpool.tile([128, DZ], f32r, tag=f"bt{k}", name=f"bt{k}") for k in range(4)]
            id_sb = cpool.tile([128, 128], f32, tag="id")
            ut8 = cpool.tile([128, 2 * UPAD], i8, tag="ut8", name="ut8")
            ut_sb = [dpool.tile([128, UPAD], fp16, tag=f"ut{k}", name=f"ut{k}") for k in range(2)]
            for k in range(4):
                nc.sync.dma_start(at_sb[k][:], ag["at"][128 * k:128 * (k + 1), :])
                nc.sync.dma_start(bth[k][:], ag["bt"][128 * k:128 * (k + 1), :])
            for k in range(2):
                nc.sync.dma_start(ct_sb[k][:], ag["ct"][128 * k:128 * (k + 1), :])
                nc.sync.dma_start(ut8[:, UPAD * k:UPAD * (k + 1)],
                                  ut_d[128 * k:128 * (k + 1), :])
            nc.sync.dma_start(id_sb[:], ag["id"][:])
            for k in range(4):
                nc.vector.tensor_copy(bt_sb[k][:], bth[k][:])   # fp16 -> f32
            for k in range(2):
                nc.vector.tensor_copy(ut_sb[k][:], ut8[:, UPAD * k:UPAD * (k + 1)])

            # drive rows (transposed): dt[m] holds drive.T[128m:128(m+1), :]
            dt_sb = [dpool.tile([128, UPAD], f32, tag=f"dt{m}", name=f"dt{m}") for m in range(4)]
            for nb in range((UPAD + 511) // 512):
                nb0 = nb * 512
                w = min(512, UPAD - nb0)
                for m in range(4):
                    psd = pp.tile([128, 512], f32, tag="ps", name=f"psD{nb}_{m}")
                    for kk in range(2):
                        nc.tensor.matmul(
                            psd[:, :w],
                            ct_sb[kk][:, 128 * m:128 * (m + 1)],
                            ut_sb[kk][:, nb0:nb0 + w],
                            start=(kk == 0), stop=(kk == 1))
                    nc.any.tensor_copy(dt_sb[m][:, nb0:nb0 + w], psd[:, :w])

            # ---- phase A: zero-init scan over NCH chunks ----
            bmat = [cpool.tile([128, NCH], f32r, tag=f"bm{m}", name=f"bm{m}") for m in range(4)]
            st_prev = []
            for m in range(4):
                t0 = stpool.tile([128, NCH], f32r, tag=f"st{m}", name=f"st0_{m}")
                nc.vector.tensor_copy(t0[:], dt_sb[m][:, 0:16 * NCH:16])
                st_prev.append(t0)
            for k in range(1, S):
                psl = [pp.tile([128, NCH], f32, tag="ps", name=f"psA{k}_{_m}") for _m in range(4)]
                for m in range(4):
                    for kk in range(4):
                        nc.tensor.matmul(
                            psl[m][:],
                            at_sb[kk][:, 128 * m:128 * (m + 1)],
                            st_prev[kk][:],
                            start=(kk == 0), stop=(kk == 3))
                st_new = []
                for m in range(4):
                    dst = (bmat[m] if k == S - 1 else
                           stpool.tile([128, NCH], f32r, tag=f"st{m}", name=f"stA{k}_{m}"))
                    nc.vector.tensor_tensor(
                        dst[:], psl[m][:],
                        dt_sb[m][:, k:k + 16 * (NCH - 1) + 1:16],
                        op=mybir.AluOpType.add)
                    st_new.append(dst)
                st_prev = st_new

            # ---- device-side tap matrices: G^(16p), G = A.T, via squaring ----
            def mat_t(src, dst, tg):      # dst = src.T
                for k in range(4):
                    for m in range(4):
                        pst = pp.tile([128, 128], f32, tag="ps", name=f"pT{tg}_{k}_{m}")
                        nc.tensor.transpose(
                            pst[:], src[k][:, 128 * m:128 * (m + 1)].bitcast(f32), id_sb[:])
                        nc.any.tensor_copy(dst[m][:, 128 * k:128 * (k + 1)], pst[:])

            def mat_mul(xT, y, dst, tg):  # dst = X @ Y  (xT = row-tiles of X.T)
                for m in range(4):
                    ps = pp.tile([128, DZ], f32, tag="ps", name=f"pM{tg}_{m}")
                    for k in range(4):
                        nc.tensor.matmul(
                            ps[:],
                            xT[k][:, 128 * m:128 * (m + 1)],
                            y[k][:],
                            start=(k == 0), stop=(k == 3))
                    nc.any.tensor_copy(dst[m][:], ps[:])

            # three rotating 512x512 buffers: px = transpose scratch,
            # py = current power, pz = G^16 (after the squaring chain)
            px = [cpool.tile([128, DZ], f32r, tag=f"px{m}", name=f"px{m}") for m in range(4)]
            py = [cpool.tile([128, DZ], f32r, tag=f"py{m}", name=f"py{m}") for m in range(4)]
            pz = [cpool.tile([128, DZ], f32r, tag=f"pz{m}", name=f"pz{m}") for m in range(4)]

            mat_t(at_sb, px, "a")          # px = A row-tiles (= G.T)
            mat_mul(px, at_sb, py, "g2")   # py = G^2
            mat_t(py, px, "t2")
            mat_mul(px, py, pz, "g4")      # pz = G^4
            mat_t(pz, px, "t4")
            mat_mul(px, pz, py, "g8")      # py = G^8
            mat_t(py, px, "t8")
            mat_mul(px, py, pz, "g16")     # pz = G^16 (kept for the chain)

            # ---- phase B: banded combine  w_c = sum_p (A^16)^p b_{c-1-p} ----
            w_prev = []
            for m in range(4):
                wt = stpool.tile([128, BCH], f32r, tag=f"w{m}", name=f"w0_{m}")
                nc.vector.tensor_copy(wt[:], bmat[m][:, K - 1:K - 1 + BCH].bitcast(f32))
                w_prev.append(wt)
            pcur = pz
            for p in range(1, K):
                if p > 1:
                    mat_t(pcur, px, f"tp{p}")
                    mat_mul(px, pz, py, f"pp{p}")
                    pcur = py
                lo = K - 1 - p
                w_new = []
                for m in range(4):
                    ps = pp.tile([128, BCH], f32, tag="ps", name=f"psW{p}_{m}")
                    for kk in range(4):
                        nc.tensor.matmul(
                            ps[:],
                            pcur[kk][:, 128 * m:128 * (m + 1)],
                            bmat[kk][:, lo:lo + BCH],
                            start=(kk == 0), stop=(kk == 3))
                    wt = stpool.tile([128, BCH], f32r, tag=f"w{m}", name=f"w{p}_{m}")
                    nc.vector.tensor_tensor(
                        wt[:], w_prev[m][:].bitcast(f32), ps[:], op=mybir.AluOpType.add)
                    w_new.append(wt)
                w_prev = w_new

            # ---- phase C: scan 256 chunks from w_c, fused output proj ----
            st_prev = w_prev
            for k in range(S):
                psl = [pp.tile([128, BCH], f32, tag="ps", name=f"psC{k}_{_m}") for _m in range(4)]
                for m in range(4):
                    for kk in range(4):
                        nc.tensor.matmul(
                            psl[m][:],
                            at_sb[kk][:, 128 * m:128 * (m + 1)],
                            st_prev[kk][:],
                            start=(kk == 0), stop=(kk == 3))
                st_new = []
                for m in range(4):
                    dst = stpool.tile([128, BCH], f32r, tag=f"sc{m}", name=f"stC{k}_{m}")
                    nc.vector.tensor_tensor(
                        dst[:], psl[m][:],
                        dt_sb[m][:, H + k:H + k + 16 * (BCH - 1) + 1:16],
                        op=mybir.AluOpType.add)
                    st_new.append(dst)
                st_prev = st_new
                # output rows t = 16*c + k for all 256 chunks c
                for h in range(2):
                    pso = pp.tile([128, DZ], f32, tag="ps", name=f"psO{k}_{h}")
                    for kk in range(4):
                        nc.tensor.matmul(
                            pso[:],
                            st_new[kk][:, 128 * h:128 * (h + 1)],
                            bt_sb[kk][:],
                            start=(kk == 0), stop=(kk == 3))
                    ob = opool.tile([128, DZ], fp16, tag="ob", name=f"ob{k}_{h}")
                    nc.any.tensor_copy(ob[:], pso[:])
                    r0 = 2048 * h + k
                    nc.sync.dma_start(out_d[r0:r0 + 2033:16, :], ob[:])
    nc.compile()
    return nc


def _build():
    if "nc" not in _CACHE:
        nc = bacc.Bacc("TRN2", target_bir_lowering=False, debug=False,
                       num_devices=NCORE)
        _CACHE["nc"] = _emit(nc)
    return _CACHE["nc"]


def _make_exec(nc):
    """Minimal replication of run_bass_via_pjrt. The output buffers are
    created on device ONCE and reused without donation -- the kernel writes
    every output element, so no host zeros ever cross the tunnel."""
    import functools
    import jax
    import jax.numpy as jnp
    from jax.sharding import Mesh, PartitionSpec, NamedSharding
    from jax.experimental.shard_map import shard_map
    from concourse import bass2jax as b2j

    b2j.install_neuronx_cc_hook()

    partition_name = nc.partition_id_tensor.name if nc.partition_id_tensor else None
    in_names, out_names, out_avals = [], [], []
    for alloc in nc.m.functions[0].allocations:
        if not isinstance(alloc, mybir.MemoryLocationSet):
            continue
        name = alloc.memorylocations[0].name
        if alloc.kind == "ExternalInput":
            if name != partition_name:
                in_names.append(name)
        elif alloc.kind == "ExternalOutput":
            shape = tuple(alloc.tensor_shape)
            dtype = mybir.dt.np(alloc.dtype)
            out_names.append(name)
            out_avals.append(jax.core.ShapedArray(shape, dtype))
    n_params = len(in_names)
    all_in = tuple(in_names + out_names + ([partition_name] if partition_name else []))

    def _body(*args):
        operands = list(args)
        if partition_name:
            operands.append(b2j.partition_id_tensor())
        outs = b2j._bass_exec_p.bind(
            *operands,
            out_avals=tuple(out_avals),
            in_names=all_in,
            out_names=tuple(out_names),
            lowering_input_output_aliases=(),
            sim_require_finite=True,
            sim_require_nnan=True,
            nc=nc,
        )
        return tuple(outs)

    devices = jax.devices()[:NCORE]
    mesh = Mesh(np.asarray(devices), ("core",))
    sharded = jax.jit(
        shard_map(
            _body, mesh=mesh,
            in_specs=(PartitionSpec("core"),) * (n_params + len(out_names)),
            out_specs=(PartitionSpec("core"),) * len(out_names),
            check_rep=False),
        keep_unused=True)

    shd = NamedSharding(mesh, PartitionSpec("core"))
    obufs = [
        jax.jit(functools.partial(
            jnp.zeros, (NCORE * a.shape[0],) + tuple(a.shape[1:]), a.dtype),
            out_shardings=shd)()
        for a in out_avals
    ]
    dbg_name = nc.dbg_addr.name if nc.dbg_addr is not None else None
    return {"sharded": sharded, "in_names": in_names, "out_names": out_names,
            "out_avals": out_avals, "obufs": obufs, "dbg_name": dbg_name}


def _get_state():
    if "exec" not in _CACHE:
        _CACHE["exec"] = _make_exec(_build())
    return _CACHE["exec"]


def _host_prep(inputs_np, A, B, C):
    """Per-run host prep: global (concatenated-over-cores) input arrays.
    u is absmax-quantized to int8; the scale folds into C.T."""
    s = float(np.abs(inputs_np).max()) / 127.0
    q = np.rint(inputs_np * (1.0 / s)).astype(np.int8)      # (T, DU)
    qT = np.ascontiguousarray(q.T)                           # (DU, T)
    ut_g = np.zeros((NCORE * DU, UPAD), np.int8)
    for i in range(NCORE):
        if i == 0:
            ut_g[:DU, H:ULEN] = qT[:, :TLOC]
        else:
            lo = i * TLOC - H
            ut_g[i * DU:(i + 1) * DU, :ULEN] = qT[:, lo:lo + TLOC + H]
    feed = {
        "ut": ut_g,
        "at": np.ascontiguousarray(A.T, dtype=np.float32),
        "ct": (np.ascontiguousarray(C.T) * np.float32(s)).astype(np.float16),
        "bt": np.ascontiguousarray(B.T).astype(np.float16),
        "id": np.eye(128, dtype=np.float32),
    }
    return feed


def _run(state, feed):
    if state["dbg_name"] is not None:
        feed = {**feed, state["dbg_name"]: np.zeros((NCORE, 2), np.uint32)}
    args = [feed[n] for n in state["in_names"]]
    outs = state["sharded"](*args, *state["obufs"])
    return np.asarray(outs[0])        # (NCORE*TLOC, DZ) fp16


def _correction(data, mean, A, B, R):
    """Output rows 0..H-1 need the A^n z0 contribution."""
    z0 = R.astype(np.float64) @ (data[0] - mean[0]).astype(np.float64)
    zc = z0
    A64, B64 = A.astype(np.float64), B.astype(np.float64)
    corr = np.empty((H, DZ), np.float64)
    for n in range(1, H + 1):
        zc = A64 @ zc
        corr[n - 1] = B64 @ zc
    return corr.astype(np.float32)


def kernel(data, inputs, mean, A, B, C, recognition_matrix, steps=None, **kw):
    data = np.asarray(data, np.float32)
    inputs_np = np.asarray(inputs, np.float32)
    mean = np.asarray(mean, np.float32)
    A = np.asarray(A, np.float32)
    B = np.asarray(B, np.float32)
    C = np.asarray(C, np.float32)
    R = np.asarray(recognition_matrix, np.float32)

    state = _get_state()
    feed = _host_prep(inputs_np, A, B, C)
    o16 = _run(state, feed)
    out = np.add(o16, mean, dtype=np.float32)
    out[:H] += _correction(data, mean, A, B, R)
    return out
